# revision 4
# baseline (speedup 1.0000x reference)
"""Trainium2 Bass kernel for nn_Activation2d (anti-aliased activation):
   y = downsample2d(leaky_relu(upsample2d(x)))  on x [8, 64, 256, 256] fp32.

Algorithm: both resamplers are separable 1D kaiser-sinc filters, expressed as
banded matrices baked with edge-replication clamping:
  U = A X          (up along H;   A [512,256], includes ratio factor 2)
  V = U A^T        (up along W)
  L = lrelu(V)
  D = L B^T        (down along W; B [256,512])
  Y = B D          (down along H)

On the PE this is 4 matmul passes per image, alternating transposing /
standard forms so each pass's contraction lands on the partition axis:
  P1 (contract h):  out1 = X^T-conv     lhsT = X        rhs = A^T   -> [w, n]
  P2 (contract w):  out2                lhsT = A^T      rhs = out1  -> [w', n]
  lrelu fused into PSUM->SBUF copy on ACT (Prelu, alpha=0.2)
  P3 (contract w'): out3                lhsT = L        rhs = B^T   -> [n, m]
  P4 (contract n):  out4                lhsT = B^T      rhs = out3  -> [h'', m]

dtypes: P1/P2/P4 in float32r (fp32 with ~12-bit mantissa, full PE speed at
free-dim >= 256), P3 in fp16 (band-limited streams < 256 would put fp32r at
4x cost). Accumulation is always fp32 in PSUM.

Sharding: pure data parallel over batch — core b computes x[b] [64,256,256].
"""
import math
from contextlib import ExitStack

import numpy as np
import ml_dtypes

import concourse.bass as bass
import concourse.bacc as bacc
import concourse.tile as tile
import concourse.mybir as mybir
from concourse.bass_utils import run_bass_kernel_spmd

RATIO = 2
KSIZE = 12
SLOPE = 0.2
H = W = 256
NCORES = 8

F32R = mybir.dt.float32r
F16 = mybir.dt.float16
F32 = mybir.dt.float32


# ----------------------------------------------------------------------------
# filter construction (mirrors the reference's kaiser_sinc_filter1d)
# ----------------------------------------------------------------------------
def _kaiser_sinc_filter1d(cutoff, half_width, kernel_size):
    half_size = kernel_size // 2
    delta_f = 4.0 * half_width
    A = 2.285 * (half_size - 1) * math.pi * delta_f + 7.95
    if A > 50.0:
        beta = 0.1102 * (A - 8.7)
    elif A >= 21.0:
        beta = 0.5842 * (A - 21.0) ** 0.4 + 0.07886 * (A - 21.0)
    else:
        beta = 0.0
    window = np.kaiser(kernel_size, beta)
    if kernel_size % 2 == 0:
        time = np.arange(-half_size, half_size) + 0.5
    else:
        time = np.arange(kernel_size) - half_size
    filt = 2.0 * cutoff * window * np.sinc(2.0 * cutoff * time)
    filt = filt / filt.sum()
    return filt.astype(np.float32)


def build_A(n_in=H):
    f = _kaiser_sinc_filter1d(0.5 / RATIO, 0.6 / RATIO, KSIZE).astype(np.float64)
    A = np.zeros((2 * n_in, n_in), np.float64)
    for t in range(n_in):
        for j in range(6):
            A[2 * t, np.clip(t + j - 3, 0, n_in - 1)] += 2.0 * f[2 * j]
            A[2 * t + 1, np.clip(t + j - 2, 0, n_in - 1)] += 2.0 * f[2 * j + 1]
    return A.astype(np.float32)


def build_B(n_out=H):
    f = _kaiser_sinc_filter1d(0.5 / RATIO, 0.6 / RATIO, KSIZE).astype(np.float64)
    B = np.zeros((n_out, 2 * n_out), np.float64)
    for m in range(n_out):
        for k in range(KSIZE):
            B[m, np.clip(2 * m + k - 5, 0, 2 * n_out - 1)] += f[k]
    return B.astype(np.float32)


def _nz_cols(mat, even=False):
    """[lo, hi) column range containing all nonzeros of mat.
    even=True rounds outward to even offsets/counts (fp32r matmul ISA rule:
    src/dst free dims must be even-count, 8B-aligned)."""
    nz = np.nonzero(np.any(mat != 0.0, axis=0))[0]
    lo, hi = int(nz[0]), int(nz[-1]) + 1
    if even:
        lo -= lo % 2
        hi += hi % 2
    return lo, hi


# ----------------------------------------------------------------------------
# bass program
# ----------------------------------------------------------------------------
def build_nc(n_img=64, dma_cast=True):
    A = build_A()          # [512, 256]
    B = build_B()          # [256, 512]
    AT = A.T.copy()        # [256, 512] rows h, cols n
    BT = B.T.copy()        # [512, 256] rows n/w', cols m/h''

    # P1: rhs windows per h-block (cols of A^T rows blk)
    p1_win = [_nz_cols(AT[128 * b:128 * (b + 1)], even=True) for b in range(2)]
    # coverage check: windows must cover all 512 cols
    assert p1_win[0][0] == 0 and p1_win[1][1] == 512 and p1_win[1][0] < p1_win[0][1]
    # P2: nonzero (q, b) blocks of lhsT = A^T[w-blk b, w'-tile q]
    p2_blocks = [
        [b for b in range(2)
         if np.any(AT[128 * b:128 * (b + 1), 128 * q:128 * (q + 1)] != 0.0)]
        for q in range(4)
    ]
    # P3: rhs m-windows per w'-block (cols of B^T rows blk)
    p3_win = [_nz_cols(BT[128 * k:128 * (k + 1)]) for k in range(4)]
    cov = np.zeros(256, bool)
    for lo, hi in p3_win:
        cov[lo:hi] = True
    assert cov.all()
    # P4: nonzero (t, k) blocks of lhsT = B^T[n-blk k, h''-tile t]
    p4_blocks = [
        [k for k in range(4)
         if np.any(BT[128 * k:128 * (k + 1), 128 * t:128 * (t + 1)] != 0.0)]
        for t in range(2)
    ]

    nc = bacc.Bacc("TRN2", target_bir_lowering=False, debug=False,
                   num_devices=NCORES)
    x_ap = nc.dram_tensor("x", [n_img, H, W], F32, kind="ExternalInput").ap()
    y_ap = nc.dram_tensor("y", [n_img, H, W], F32, kind="ExternalOutput").ap()

    at_dram = nc.inline_tensor(AT.astype(np.float32), name="at_f32")
    bt16_dram = nc.inline_tensor(BT.astype(np.float16), name="bt_f16")
    bt32_dram = nc.inline_tensor(BT.astype(np.float32), name="bt_f32")

    with tile.TileContext(nc) as tc, ExitStack() as ctx:
        cpool = ctx.enter_context(tc.tile_pool(name="consts", bufs=1))
        xpool = ctx.enter_context(tc.tile_pool(name="xin", bufs=6))
        upool = ctx.enter_context(tc.tile_pool(name="u", bufs=6))
        lpool = ctx.enter_context(tc.tile_pool(name="l", bufs=10))
        dpool = ctx.enter_context(tc.tile_pool(name="d", bufs=6))
        opool = ctx.enter_context(tc.tile_pool(name="o", bufs=3))
        pp1 = ctx.enter_context(tc.tile_pool(name="pp1", bufs=2, space="PSUM"))
        pp2 = ctx.enter_context(tc.tile_pool(name="pp2", bufs=3, space="PSUM"))
        pp3 = ctx.enter_context(tc.tile_pool(name="pp3", bufs=2, space="PSUM"))
        pp4 = ctx.enter_context(tc.tile_pool(name="pp4", bufs=1, space="PSUM"))

        # ---- constants: stage fp32, round on-chip to fp32r -------------
        ATr, BT16, BTr = [], [], []
        for b in range(2):
            stg = cpool.tile([128, 512], F32, tag=f"at_stg{b}")
            nc.sync.dma_start(stg[:], at_dram.ap()[128 * b:128 * (b + 1), :])
            t = cpool.tile([128, 512], F32R, tag=f"at_r{b}")
            nc.vector.tensor_copy(t[:], stg[:])
            ATr.append(t)
        for k in range(4):
            t16 = cpool.tile([128, 256], F16, tag=f"bt16_{k}")
            nc.sync.dma_start(t16[:], bt16_dram.ap()[128 * k:128 * (k + 1), :])
            BT16.append(t16)
            stg = cpool.tile([128, 256], F32, tag=f"bt_stg{k}")
            nc.sync.dma_start(stg[:], bt32_dram.ap()[128 * k:128 * (k + 1), :])
            tr = cpool.tile([128, 256], F32R, tag=f"bt_r{k}")
            nc.vector.tensor_copy(tr[:], stg[:])
            BTr.append(tr)

        # ---- per-image pipeline ----------------------------------------
        for c in range(n_img):
            # load X (cast fp32 -> fp32r during DMA if supported, else via DVE)
            Xr = []
            for b in range(2):
                if dma_cast:
                    xr = xpool.tile([128, 256], F32R, tag="xr")
                    nc.gpsimd.dma_start(xr[:], x_ap[c, 128 * b:128 * (b + 1), :])
                else:
                    xf = xpool.tile([128, 256], F32, tag="xf")
                    nc.sync.dma_start(xf[:], x_ap[c, 128 * b:128 * (b + 1), :])
                    xr = xpool.tile([128, 256], F32R, tag="xr")
                    nc.vector.tensor_copy(xr[:], xf[:])
                Xr.append(xr)

            # P1: out1[w-blk] [128, 512] = sum_h X[h, w-blk] A^T[h, n]
            U = []
            for b_out in range(2):
                ps = pp1.tile([128, 512], F32, tag="ps1")
                for i, b in enumerate(range(2)):
                    lo, hi = p1_win[b]
                    nc.tensor.matmul(
                        ps[:, lo:hi],
                        Xr[b][:, 128 * b_out:128 * (b_out + 1)],
                        ATr[b][:, lo:hi],
                        start=(i == 0), stop=(i == 1),
                    )
                u = upool.tile([128, 512], F32R, tag="u")
                nc.vector.tensor_copy(u[:], ps[:])
                U.append(u)

            # P2: out2[w'-tile q] [128, 512] = sum_w A^T[w, w'-q] out1[w, n]
            L = []
            for q in range(4):
                ps = pp2.tile([128, 512], F32, tag="ps2")
                blocks = p2_blocks[q]
                for i, b in enumerate(blocks):
                    nc.tensor.matmul(
                        ps[:],
                        ATr[b][:, 128 * q:128 * (q + 1)],
                        U[b][:],
                        start=(i == 0), stop=(i == len(blocks) - 1),
                    )
                l = lpool.tile([128, 512], F16, tag="l")
                nc.scalar.activation(l[:], ps[:],
                                     mybir.ActivationFunctionType.Prelu,
                                     alpha=SLOPE)
                L.append(l)

            # P3: out3 psum group g holds n-blks (2g, 2g+1) side by side
            D = []
            for g in range(2):
                ps = pp3.tile([128, 512], F32, tag="ps3")
                first = True
                for j, nb in enumerate((2 * g, 2 * g + 1)):
                    for k in range(4):
                        lo, hi = p3_win[k]
                        nc.tensor.matmul(
                            ps[:, 256 * j + lo:256 * j + hi],
                            L[k][:, 128 * nb:128 * (nb + 1)],
                            BT16[k][:, lo:hi],
                            start=first,
                            stop=(j == 1 and k == 3),
                        )
                        first = False
                d = dpool.tile([128, 512], F32R, tag="d")
                nc.vector.tensor_copy(d[:], ps[:])
                D.append(d)

            # P4: out4 [128, 512]: h''-tile t in cols 256t..256t+256
            ps4 = pp4.tile([128, 512], F32, tag="ps4")
            first = True
            for t in range(2):
                blocks = p4_blocks[t]
                for i, k in enumerate(blocks):
                    nc.tensor.matmul(
                        ps4[:, 256 * t:256 * (t + 1)],
                        BTr[k][:, 128 * t:128 * (t + 1)],
                        D[k // 2][:, 256 * (k % 2):256 * (k % 2 + 1)],
                        start=first,
                        stop=(t == 1 and i == len(blocks) - 1),
                    )
                    first = False
            o = opool.tile([128, 512], F32, tag="o")
            nc.scalar.copy(o[:], ps4[:])
            nc.sync.dma_start(
                y_ap[c].rearrange("(t p) w -> p t w", p=128),
                o[:].rearrange("p (t w) -> p t w", t=2))

    nc.compile()
    return nc


_NC_CACHE = {}


def _get_nc(n_img, dma_cast=True):
    key = (n_img, dma_cast)
    if key not in _NC_CACHE:
        _NC_CACHE[key] = build_nc(n_img, dma_cast)
    return _NC_CACHE[key]


def kernel(x: np.ndarray) -> np.ndarray:
    """x: [8, 64, 256, 256] fp32 -> y same shape."""
    x = np.asarray(x, dtype=np.float32)
    assert x.shape == (NCORES, 64, H, W), x.shape
    nc = _get_nc(64)
    in_maps = [{"x": x[b]} for b in range(NCORES)]
    res = run_bass_kernel_spmd(nc, in_maps, core_ids=list(range(NCORES)))
    return np.stack([res.results[b]["y"] for b in range(NCORES)], axis=0)
